# revision 1
# baseline (speedup 1.0000x reference)
"""Trainium2 Bass kernel for nn_DifferentiableCDF (soft Gaussian histogram -> CDF).

Algorithm (per core, data-parallel over pixels; 12 units x 8192 px each):
  u = 255*x in bin units; hi = floor(u/16) (16-bin block), d = u/16 - hi in [0,1].
  Gaussian weight for bin j = 16*(hi-1) + w (w in [W_LO, W_HI)) is
    exp(-ALPHAP*(d + c_w)^2),  c_w = (16-w)/16,  ALPHAP = 256/(255*sigma)^2
  (truncated support: |u - j| > ~10 bins contributes < 1e-6 relative).
  Device pipeline: DVE builds dm = d + c_w per column (bf16 4x tensor_scalar)
  and arg = dm*dm (bf16 2x tensor_tensor), ACT applies one bulk Exp per pixel
  group into fp16 weights, and the TensorE scatters each 128-pixel chunk with
  a 16-wide one-hot-of-hi matmul accumulating into a per-(unit, hi) PSUM
  table.  Host folds the 16 x 34-column block tables into the 256-bin
  histogram (the only cross-core reduction), normalizes, and cumsums.
"""
import sys
if "/opt/trn_rl_repo" not in sys.path:
    sys.path.insert(0, "/opt/trn_rl_repo")

import numpy as np
from concourse import bacc, tile
from concourse.bass_utils import run_bass_kernel_spmd
import concourse.mybir as mybir

# ---- problem constants (hardcoded per spec) ----
B, C, H, W = 4, 3, 256, 256
UNITS = B * C                  # 12 independent histograms
NPIX = H * W                   # 65536 pixels per unit
NCORES = 8
PIX_PER_CORE = NPIX // NCORES  # 8192 pixels per unit per core
CHUNKS_PER_UNIT = PIX_PER_CORE // 128  # 64
NCHUNK = UNITS * CHUNKS_PER_UNIT       # 768 chunks of 128 pixels
SIGMA = 0.01
BINS = 256
ALPHAP = 256.0 / (255.0 * SIGMA) ** 2  # 39.369...
W_LO, W_HI = 7, 41
NW = W_HI - W_LO                        # 34 columns
# column classes:
#  simple  (w in [8,16)):  DVE STT arg = d*(d+2c), host descale exp(-a'c^2)
#  centered-DVE:           dm = d+c (tensor_scalar), arg = dm*dm (STT)
#  centered-ACT:           arg = Square(d + c) on ACT
SIMPLE_COLS = set(range(8, 16))
ACT_COLS = set(range(30, 41))
NGROUP = 3                              # pixel-chunk pipeline groups
GCHUNK = NCHUNK // NGROUP               # 256 chunks per group
DT = mybir.dt

_COMPILED = None  # cached (nc, meta)


def _emit_body(nc, tc, pool, pipe, psum_pool, x_ext, tbl_ext, emit_cols=True, emit_mm=True):
    xc = pool.tile([128, NCHUNK], DT.float32)
    nc.sync.dma_start(xc[:], x_ext[:])

    hi_i = pool.tile([128, NCHUNK], DT.int32)
    hi_f = pool.tile([128, NCHUNK], DT.float32)
    d_b = pool.tile([128, NCHUNK], DT.bfloat16)

    # hi = floor(x*15.9375) via RNE(x*15.9375 - 0.5) [HW converts RNE]
    nc.vector.tensor_scalar(hi_i[:], xc[:], 15.9375, -0.5,
                            mybir.AluOpType.mult, mybir.AluOpType.add)
    nc.vector.tensor_copy(hi_f[:], hi_i[:])
    # d = x*15.9375 - hi  in [0, 1]  (bf16)
    nc.vector.scalar_tensor_tensor(d_b[:], xc[:], 15.9375, hi_f[:],
                                   mybir.AluOpType.mult,
                                   mybir.AluOpType.subtract)

    oh = pool.tile([128, 16, NCHUNK], DT.float16)

    # dm[p, wi, cc] = d_b[p, cc] + (16 - w)/16, one 4x-mode ts_add per column
    dm = pool.tile([128, NW, NCHUNK], DT.bfloat16)
    if emit_cols:
        for wi in range(NW):
            nc.vector.tensor_scalar(dm[:, wi, :], d_b[:],
                                    (16.0 - (W_LO + wi)) / 16.0, None,
                                    mybir.AluOpType.add)

    acc = psum_pool.tile([16, UNITS * NW], DT.float32)

    for g in range(NGROUP):
        c0 = g * GCHUNK
        if emit_cols:
            # arg = dm*dm (bf16 2x), then weights = exp(-ALPHAP*arg) in fp16
            arg = pipe.tile([128, NW, GCHUNK], DT.bfloat16, tag="arg")
            bt_ = pipe.tile([128, NW, GCHUNK], DT.float16, tag="bwt")
            dmg = dm[:, :, c0:c0 + GCHUNK]
            nc.vector.tensor_tensor(arg[:], dmg, dmg, mybir.AluOpType.mult)
            nc.scalar.activation(bt_[:], arg[:],
                                 mybir.ActivationFunctionType.Exp,
                                 scale=-ALPHAP)
        else:
            bt_ = pipe.tile([128, NW, GCHUNK], DT.float16, tag="bwt")
        if g == 0:
            # one-hot of hi emitted here so it overlaps ACT's exp of group 0
            for m in range(16):
                nc.vector.tensor_scalar(oh[:, m, :], hi_f[:], float(m), None,
                                        mybir.AluOpType.is_equal)
        if emit_mm:
            for cc in range(GCHUNK):
                c = c0 + cc
                t, j = divmod(c, CHUNKS_PER_UNIT)
                nc.tensor.matmul(acc[:, t * NW:(t + 1) * NW],
                                 oh[:, :, c], bt_[:, :, cc],
                                 start=(j == 0),
                                 stop=(j == CHUNKS_PER_UNIT - 1))

    out_sb = pool.tile([16, UNITS * NW], DT.float32)
    if emit_mm:
        nc.vector.tensor_copy(out_sb[:], acc[:])
    else:
        nc.vector.tensor_copy(out_sb[:], oh[0:16, 0, 0:UNITS * NW])
    nc.sync.dma_start(tbl_ext[:], out_sb[:])


def _build(loop_n=1, emit_cols=True, emit_mm=True):
    nc = bacc.Bacc("TRN2", target_bir_lowering=False, debug=False,
                   num_devices=NCORES)
    x_ext = nc.declare_dram_parameter("xc", [128, NCHUNK], DT.float32,
                                      isOutput=False)
    tbl_ext = nc.declare_dram_parameter("table", [16, UNITS * NW], DT.float32,
                                        isOutput=True)

    with tile.TileContext(nc) as tc:
        with (
            tc.tile_pool(name="pool", bufs=1) as pool,
            tc.tile_pool(name="pipe", bufs=2) as pipe,
            tc.tile_pool(name="psum", bufs=1, space="PSUM") as psum_pool,
        ):
            if loop_n == 1:
                _emit_body(nc, tc, pool, pipe, psum_pool, x_ext, tbl_ext, emit_cols, emit_mm)
            else:
                engs = [mybir.EngineType.PE, mybir.EngineType.DVE,
                        mybir.EngineType.Activation, mybir.EngineType.SP,
                        mybir.EngineType.Pool]
                with tc.For_i(0, loop_n, 1, hint_engines=engs):
                    _emit_body(nc, tc, pool, pipe, psum_pool, x_ext, tbl_ext, emit_cols, emit_mm)

    nc.compile()
    return nc


def _get_compiled():
    global _COMPILED
    if _COMPILED is None:
        _COMPILED = _build()
    return _COMPILED


def _shard_x(x):
    """x (B,C,H,W) -> per-core [128, NCHUNK] arrays; element [p, 64t+j] =
    unit t, pixel 8192*core + 128*j + p."""
    xu = np.ascontiguousarray(x.reshape(UNITS, NPIX))
    shards = []
    for core in range(NCORES):
        sl = xu[:, core * PIX_PER_CORE:(core + 1) * PIX_PER_CORE]
        # (UNITS, 64, 128) -> (128, UNITS, 64)
        sl = sl.reshape(UNITS, CHUNKS_PER_UNIT, 128).transpose(2, 0, 1)
        shards.append(np.ascontiguousarray(sl.reshape(128, NCHUNK), np.float32))
    return shards


def _postprocess(tables):
    """tables: list of NCORES arrays [16, UNITS*NW] -> cdf (B, C, BINS) fp32."""
    tab = np.zeros((16, UNITS, NW), np.float64)
    for t in tables:
        tab += t.reshape(16, UNITS, NW).astype(np.float64)
    # descale simple-path columns by exp(-ALPHAP*c^2)
    ws = np.arange(W_LO, W_HI)
    cw = (16.0 - ws) / 16.0
    # all columns are centered (d+c)^2 form: no descale needed
    # fold: bin j = 16*(J-1) + w
    hist = np.zeros((UNITS, 16 + BINS + 48), np.float64)
    for J in range(16):
        hist[:, 16 * J + W_LO: 16 * J + W_HI] += tab[J, :, :]
    hist = hist[:, 16:16 + BINS]
    pdf = hist / (hist.sum(-1, keepdims=True) + 1e-6)
    cdf = np.cumsum(pdf, -1)
    return cdf.reshape(B, C, BINS).astype(np.float32)


def run_device(x, trace=False):
    nc = _get_compiled()
    in_maps = [{"xc": s} for s in _shard_x(np.asarray(x))]
    res = run_bass_kernel_spmd(nc, in_maps, list(range(NCORES)), trace=trace)
    tables = [res.results[i]["table"] for i in range(NCORES)]
    return tables, res


def kernel(x, centers):
    # centers is linspace(0,1,256) by construction; bin geometry is hardcoded.
    tables, _ = run_device(x)
    return _postprocess(tables)


if __name__ == "__main__":
    import jax, jax.numpy as jnp
    key = jax.random.key(0)
    k1, _ = jax.random.split(key)
    x = np.asarray(jax.random.uniform(k1, (B, C, H, W), dtype=jnp.float32))
    centers = np.linspace(0, 1, BINS, dtype=np.float32)
    out = kernel(x, centers)
    print("kernel output", out.shape, out.dtype, out[0, 0, :5], out[0, 0, -1])



# revision 2
# speedup vs baseline: 1.9833x; 1.9833x over previous
"""Trainium2 Bass kernel for nn_DifferentiableCDF (soft Gaussian histogram -> CDF).

Algorithm v2 — exact integer-bin histogram on device, Gaussian on host:
  The Gaussian soft-binning weight exp(-((255x - j)/2.55)^2) varies slowly
  enough that quantizing each pixel to its nearest integer bin m = RNE(255x)
  and applying the Gaussian as a 256x256 host-side table multiply changes the
  final CDF by < 2e-4 (vs 2e-2 tolerance).  So the device kernel only builds
  an EXACT 256-bin histogram per (B,C) unit, factored as a 16x16 joint count
  table F[hi, s] (hi = m>>4, s = m&15) via a one-hot x one-hot matmul:
    - ACT: m = RNE(255*x + 1024) in fp16 (fp16 is integer-exact in [1024,2048))
    - DVE: hi = RNE(m/16 - 64.4995) (int32), s = m - 16*hi (+1024 offset)
    - DVE: one-hots of hi (16 wide) and s (16 wide) via two broadcast
      tensor_tensor is_equal ops against constant index tiles (4x mode)
    - PE:  per 128-pixel chunk, matmul oh_hi^T @ oh_s accumulates the 16x16
      count table in PSUM; 4-way column tiling (tile_position) runs 4 chunks
      concurrently in different 32-col strips of the PE array.
  PSUM trick: the bank is DVE-memset to zero and all matmuls use start=False
  so per-element has_written semantics accumulate correctly without bank-wide
  clears (which would corrupt interleaved accumulation groups).
  Host folds 8 cores x 4 strips, applies the Gaussian table, normalizes and
  cumsums (output is only 12x256).
"""
import sys
if "/opt/trn_rl_repo" not in sys.path:
    sys.path.insert(0, "/opt/trn_rl_repo")

import numpy as np
from concourse import bacc, tile
from concourse.bass_utils import run_bass_kernel_spmd
import concourse.mybir as mybir

# ---- problem constants (hardcoded per spec) ----
B, C, H, W = 4, 3, 256, 256
UNITS = B * C                  # 12 independent histograms
NPIX = H * W                   # 65536 pixels per unit
NCORES = 8
PIX_PER_CORE = NPIX // NCORES  # 8192 pixels per unit per core
CHUNKS_PER_UNIT = PIX_PER_CORE // 128  # 64
NCHUNK = UNITS * CHUNKS_PER_UNIT       # 768 chunks of 128 pixels
SIGMA = 0.01
BINS = 256
ALPHA = 1.0 / (255.0 * SIGMA) ** 2     # Gaussian exponent per bin^2
NHALF = NCHUNK // 2                    # is_equal split granularity
DT = mybir.dt

_COMPILED = None  # cached nc


def _emit_body(nc, tc, pool, pipe, psum_pool, const_hi, const_s, x_ext, tbl_ext,
               emit_oh=True, emit_mm=True):
    xc = pipe.tile([128, NCHUNK], DT.float32, tag="xc")
    nc.sync.dma_start(xc[:], x_ext[:])

    m_h = pipe.tile([128, NCHUNK], DT.float16, tag="mh")
    hi_i = pipe.tile([128, NCHUNK], DT.int32, tag="hii")
    hi_h = pipe.tile([128, NCHUNK], DT.float16, tag="hih")
    s_h = pipe.tile([128, NCHUNK], DT.float16, tag="sh")

    # m = RNE(255x + 1024); fp16 grid spacing is exactly 1.0 in [1024, 2048)
    nc.scalar.activation(m_h[:], xc[:], mybir.ActivationFunctionType.Copy,
                         bias=1024.0, scale=255.0)
    # hi = floor((m-1024)/16) via RNE(m/16 - 64.4995); int32 convert is RNE
    nc.vector.tensor_scalar(hi_i[:], m_h[:], 0.0625, -64.4995,
                            mybir.AluOpType.mult, mybir.AluOpType.add)
    nc.vector.tensor_copy(hi_h[:], hi_i[:])
    # s = m - 16*hi  in [1024, 1039]
    nc.vector.scalar_tensor_tensor(s_h[:], hi_h[:], -16.0, m_h[:],
                                   mybir.AluOpType.mult, mybir.AluOpType.add)

    oh_hi = pipe.tile([128, 16, NCHUNK], DT.float16, tag="ohhi")
    oh_s = pipe.tile([128, 16, NCHUNK], DT.float16, tag="ohs")
    if emit_oh:
        for h0 in range(0, NCHUNK, NHALF):
            sl = slice(h0, h0 + NHALF)
            hi_b = hi_h[:, sl].unsqueeze(1).broadcast_to((128, 16, NHALF))
            s_b = s_h[:, sl].unsqueeze(1).broadcast_to((128, 16, NHALF))
            nc.vector.tensor_tensor(oh_hi[:, :, sl], hi_b, const_hi[:, :, sl],
                                    mybir.AluOpType.is_equal)
            nc.vector.tensor_tensor(oh_s[:, :, sl], s_b, const_s[:, :, sl],
                                    mybir.AluOpType.is_equal)

    acc = psum_pool.tile([128, 512], DT.float32)  # one full PSUM bank
    nc.vector.memset(acc[:, 0:UNITS * 16], 0.0)

    if emit_mm:
        for t in range(UNITS):
            for jj in range(CHUNKS_PER_UNIT // 4):
                for g in range(4):
                    c = t * CHUNKS_PER_UNIT + jj * 4 + g
                    nc.tensor.matmul(acc[32 * g:32 * g + 16, 16 * t:16 * t + 16],
                                     oh_hi[:, :, c], oh_s[:, :, c],
                                     start=False, stop=False,
                                     skip_group_check=True,
                                     tile_position=(0, 32 * g))

    out_sb = pipe.tile([128, UNITS * 16], DT.float32, tag="osb")
    nc.scalar.activation(out_sb[:], acc[:, 0:UNITS * 16],
                         mybir.ActivationFunctionType.Copy)
    nc.sync.dma_start(tbl_ext[:], out_sb[:])


def _build(loop_n=1, emit_oh=True, emit_mm=True):
    nc = bacc.Bacc("TRN2", target_bir_lowering=False, debug=False,
                   num_devices=NCORES)
    x_ext = nc.declare_dram_parameter("xc", [128, NCHUNK], DT.float32,
                                      isOutput=False)
    tbl_ext = nc.declare_dram_parameter("table", [128, UNITS * 16], DT.float32,
                                        isOutput=True)

    with tile.TileContext(nc) as tc:
        with (
            tc.tile_pool(name="pool", bufs=1) as pool,
            tc.tile_pool(name="pipe", bufs=2) as pipe,
            tc.tile_pool(name="psum", bufs=1, space="PSUM") as psum_pool,
        ):
            # loop-invariant one-hot comparison index tiles
            const_hi = pool.tile([128, 16, NCHUNK], DT.float16)
            const_s = pool.tile([128, 16, NCHUNK], DT.float16)
            for w in range(16):
                nc.vector.memset(const_hi[:, w, :], float(w))
                nc.vector.memset(const_s[:, w, :], float(1024 + w))

            if loop_n == 1:
                _emit_body(nc, tc, pool, pipe, psum_pool, const_hi, const_s,
                           x_ext, tbl_ext, emit_oh, emit_mm)
            else:
                engs = [mybir.EngineType.PE, mybir.EngineType.DVE,
                        mybir.EngineType.Activation, mybir.EngineType.SP,
                        mybir.EngineType.Pool]
                with tc.For_i(0, loop_n, 1, hint_engines=engs):
                    _emit_body(nc, tc, pool, pipe, psum_pool, const_hi,
                               const_s, x_ext, tbl_ext, emit_oh, emit_mm)

    nc.compile()
    return nc


def _get_compiled():
    global _COMPILED
    if _COMPILED is None:
        _COMPILED = _build()
    return _COMPILED


def _shard_x(x):
    """x (B,C,H,W) -> per-core [128, NCHUNK] arrays; element [p, 64t+j] =
    unit t, pixel 8192*core + 128*j + p."""
    xu = np.ascontiguousarray(x.reshape(UNITS, NPIX))
    shards = []
    for core in range(NCORES):
        sl = xu[:, core * PIX_PER_CORE:(core + 1) * PIX_PER_CORE]
        # (UNITS, 64, 128) -> (128, UNITS, 64)
        sl = sl.reshape(UNITS, CHUNKS_PER_UNIT, 128).transpose(2, 0, 1)
        shards.append(np.ascontiguousarray(sl.reshape(128, NCHUNK), np.float32))
    return shards


_GTAB = None


def _gauss_table():
    global _GTAB
    if _GTAB is None:
        m = np.arange(BINS, dtype=np.float64)
        _GTAB = np.exp(-ALPHA * (m[:, None] - m[None, :]) ** 2)
    return _GTAB


def _postprocess(tables):
    """tables: list of NCORES arrays [128, UNITS*16] -> cdf (B, C, BINS)."""
    tab = np.zeros((128, UNITS * 16), np.float64)
    for t in tables:
        tab += t.astype(np.float64)
    # F[t, hi, s]: strip g holds partitions [32g, 32g+16)
    F = np.zeros((UNITS, 16, 16), np.float64)
    for g in range(4):
        F += tab[32 * g:32 * g + 16, :].reshape(16, UNITS, 16).transpose(1, 0, 2)
    F = F.reshape(UNITS, BINS)            # m = 16*hi + s
    hist = F @ _gauss_table()
    pdf = hist / (hist.sum(-1, keepdims=True) + 1e-6)
    cdf = np.cumsum(pdf, -1)
    return cdf.reshape(B, C, BINS).astype(np.float32)


def run_device(x, trace=False):
    nc = _get_compiled()
    in_maps = [{"xc": s} for s in _shard_x(np.asarray(x))]
    res = run_bass_kernel_spmd(nc, in_maps, list(range(NCORES)), trace=trace)
    tables = [res.results[i]["table"] for i in range(NCORES)]
    return tables, res


def kernel(x, centers):
    # centers is linspace(0,1,256) by construction; bin geometry is hardcoded.
    tables, _ = run_device(x)
    return _postprocess(tables)


if __name__ == "__main__":
    import jax, jax.numpy as jnp
    key = jax.random.key(0)
    k1, _ = jax.random.split(key)
    x = np.asarray(jax.random.uniform(k1, (B, C, H, W), dtype=jnp.float32))
    centers = np.linspace(0, 1, BINS, dtype=np.float32)
    out = kernel(x, centers)
    print("kernel output", out.shape, out.dtype, out[0, 0, :5], out[0, 0, -1])


# revision 10
# speedup vs baseline: 2.2252x; 1.1219x over previous
"""Trainium2 Bass kernel for nn_DifferentiableCDF (soft Gaussian histogram -> CDF).

Exact integer-bin histogram on device, Gaussian applied on host:
  The Gaussian soft-binning weight exp(-((255x - j)/2.55)^2) varies slowly
  enough that quantizing each pixel to its nearest integer bin m = RNE(255x)
  and applying the Gaussian as a 256x256 host-side table multiply changes the
  final CDF by < 2e-4 (vs 2e-2 tolerance).  The device kernel only builds an
  EXACT 256-bin histogram per (B,C) unit, factored as a 16x16 joint count
  table F[hi, s] (hi = m>>4, s = m&15) via one-hot x one-hot matmuls:
    - ACT: m = RNE(255*x + 1024) in fp16 (fp16 is integer-exact in [1024,2048))
    - DVE: hi = RNE(m/16 - 64.4995) (int32 RNE convert), s = m - 16*hi
    - DVE: 16-wide one-hots of hi and s via two broadcast tensor_tensor
      is_equal ops against constant index tiles (4x perf mode)
    - PE:  8 chunks (8*128 pixels) per matmul: the one-hots are laid out
      [128, cb, w=16, cc=8] so each weight block [:, cb, :, :] is 128
      contiguous columns; out[8w+cc, 8s+cc'] accumulates in PSUM, the
      cc-diagonal holds the per-chunk 16x16 tables (96 matmuls total).
  PSUM trick: banks are DVE-memset to zero and all matmuls use start=False so
  per-element has_written semantics accumulate correctly without bank-wide
  clears (which would corrupt interleaved accumulation groups).
  Host sums 8 cores, extracts the cc-diagonal, applies the Gaussian table,
  normalizes and cumsums (output is only 12x256).
"""
import sys
if "/opt/trn_rl_repo" not in sys.path:
    sys.path.insert(0, "/opt/trn_rl_repo")

import numpy as np
from concourse import bacc, tile
from concourse.bass_utils import run_bass_kernel_spmd
import concourse.mybir as mybir

# ---- problem constants (hardcoded per spec) ----
B, C, H, W = 4, 3, 256, 256
UNITS = B * C                  # 12 independent histograms
NPIX = H * W                   # 65536 pixels per unit
NCORES = 8
PIX_PER_CORE = NPIX // NCORES  # 8192 pixels per unit per core
CHUNKS_PER_UNIT = PIX_PER_CORE // 128  # 64
NCHUNK = UNITS * CHUNKS_PER_UNIT       # 768 chunks of 128 pixels
CB = NCHUNK // 8                       # 96 8-chunk superblocks
SIGMA = 0.01
BINS = 256
ALPHA = 1.0 / (255.0 * SIGMA) ** 2     # Gaussian exponent per bin^2
NB = 3                                 # PSUM banks (4 units each)
DT = mybir.dt

_COMPILED = None  # cached nc


def _emit_body(nc, tc, pool, pipe, psum_pool, const_hi, const_s, x_ext,
               tbl_ext, emit_oh=True, emit_mm=True):
    xc = pipe.tile([128, CB, 8], DT.float32, tag="xc")
    nc.sync.dma_start(xc[:], x_ext[:])

    m_h = pipe.tile([128, CB, 8], DT.float16, tag="mh")
    hi_i = pipe.tile([128, CB, 8], DT.int32, tag="hii")
    hi_h = pipe.tile([128, CB, 8], DT.float16, tag="hih")
    s_h = pipe.tile([128, CB, 8], DT.float16, tag="sh")

    # m = RNE(255x + 1024); fp16 grid spacing is exactly 1.0 in [1024, 2048)
    nc.scalar.activation(m_h[:], xc[:], mybir.ActivationFunctionType.Copy,
                         bias=1024.0, scale=255.0)
    # hi = floor((m-1024)/16) via RNE(m/16 - 64.4995); int32 convert is RNE
    nc.vector.tensor_scalar(hi_i[:], m_h[:], 0.0625, -64.4995,
                            mybir.AluOpType.mult, mybir.AluOpType.add)
    nc.vector.tensor_copy(hi_h[:], hi_i[:])
    # s = m - 16*hi  in [1024, 1039]
    nc.vector.scalar_tensor_tensor(s_h[:], hi_h[:], -16.0, m_h[:],
                                   mybir.AluOpType.mult, mybir.AluOpType.add)

    oh_hi = pipe.tile([128, CB, 16, 8], DT.float16, tag="ohhi")
    oh_s = pipe.tile([128, CB, 16, 8], DT.float16, tag="ohs")
    if emit_oh:
        hv = hi_h[:].unsqueeze(2).broadcast_to((128, CB, 16, 8))
        sv = s_h[:].unsqueeze(2).broadcast_to((128, CB, 16, 8))
        nc.vector.tensor_tensor(oh_hi[:], hv, const_hi[:],
                                mybir.AluOpType.is_equal)
        nc.vector.tensor_tensor(oh_s[:], sv, const_s[:],
                                mybir.AluOpType.is_equal)
    else:
        nc.vector.memset(oh_hi[:, 0, 0, :], 0.0)
        nc.vector.memset(oh_s[:, 0, 0, :], 0.0)

    accs = []
    for i in range(NB):
        acc_i = psum_pool.tile([128, 512], DT.float32, tag=f"acc{i}")
        accs.append(acc_i)
    for a in accs:
        nc.vector.memset(a[:], 0.0)

    if emit_mm:
        for t in range(UNITS):
            a = accs[t // 4]
            col = (t % 4) * 128
            for jj in range(CHUNKS_PER_UNIT // 8):
                cb = t * (CHUNKS_PER_UNIT // 8) + jj
                nc.tensor.matmul(a[:, col:col + 128],
                                 oh_hi[:, cb, :, :].rearrange(
                                     "p a b -> p (a b)"),
                                 oh_s[:, cb, :, :].rearrange(
                                     "p a b -> p (a b)"),
                                 start=False, stop=False,
                                 skip_group_check=True)

    out_sb = pipe.tile([128, NB * 512], DT.float32, tag="osb")
    for i, a in enumerate(accs):
        nc.scalar.activation(out_sb[:, i * 512:(i + 1) * 512], a[:],
                             mybir.ActivationFunctionType.Copy)
    nc.sync.dma_start(tbl_ext[:], out_sb[:])


def _build(loop_n=1, emit_oh=True, emit_mm=True):
    nc = bacc.Bacc("TRN2", target_bir_lowering=False, debug=False,
                   num_devices=NCORES)
    x_ext = nc.declare_dram_parameter("xc", [128, NCHUNK], DT.float32,
                                      isOutput=False)
    tbl_ext = nc.declare_dram_parameter("table", [128, NB * 512], DT.float32,
                                        isOutput=True)

    with tile.TileContext(nc) as tc:
        with (
            tc.tile_pool(name="pool", bufs=1) as pool,
            tc.tile_pool(name="pipe", bufs=2) as pipe,
            tc.tile_pool(name="psum", bufs=1, space="PSUM") as psum_pool,
        ):
            # loop-invariant one-hot comparison index tiles
            const_hi = pool.tile([128, CB, 16, 8], DT.float16)
            const_s = pool.tile([128, CB, 16, 8], DT.float16)
            for w in range(16):
                nc.vector.memset(const_hi[:, :, w, :], float(w))
                nc.vector.memset(const_s[:, :, w, :], float(1024 + w))

            if loop_n == 1:
                _emit_body(nc, tc, pool, pipe, psum_pool, const_hi, const_s,
                           x_ext, tbl_ext, emit_oh, emit_mm)
            else:
                engs = [mybir.EngineType.PE, mybir.EngineType.DVE,
                        mybir.EngineType.Activation, mybir.EngineType.SP,
                        mybir.EngineType.Pool]
                with tc.For_i(0, loop_n, 1, hint_engines=engs):
                    _emit_body(nc, tc, pool, pipe, psum_pool, const_hi,
                               const_s, x_ext, tbl_ext, emit_oh, emit_mm)

    nc.compile()
    return nc


def _get_compiled():
    global _COMPILED
    if _COMPILED is None:
        _COMPILED = _build()
    return _COMPILED


def _shard_x(x):
    """x (B,C,H,W) -> per-core [128, NCHUNK] arrays; element [p, 64t+j] =
    unit t, pixel 8192*core + 128*j + p."""
    xu = np.ascontiguousarray(x.reshape(UNITS, NPIX))
    shards = []
    for core in range(NCORES):
        sl = xu[:, core * PIX_PER_CORE:(core + 1) * PIX_PER_CORE]
        # (UNITS, 64, 128) -> (128, UNITS, 64)
        sl = sl.reshape(UNITS, CHUNKS_PER_UNIT, 128).transpose(2, 0, 1)
        shards.append(np.ascontiguousarray(sl.reshape(128, NCHUNK), np.float32))
    return shards


_GTAB = None


def _gauss_table():
    global _GTAB
    if _GTAB is None:
        m = np.arange(BINS, dtype=np.float64)
        _GTAB = np.exp(-ALPHA * (m[:, None] - m[None, :]) ** 2)
    return _GTAB


def _postprocess(tables):
    """tables: list of NCORES arrays [128, NB*512] -> cdf (B, C, BINS)."""
    tab = np.zeros((128, NB * 512), np.float64)
    for t in tables:
        tab += t.astype(np.float64)
    F = np.zeros((UNITS, 16, 16), np.float64)
    for t in range(UNITS):
        col = (t // 4) * 512 + (t % 4) * 128
        blk = tab[:, col:col + 128].reshape(16, 8, 16, 8)  # [w, cc, s, cc']
        F[t] = np.einsum('wasa->ws', blk)
    F = F.reshape(UNITS, BINS)            # m = 16*hi + s
    hist = F @ _gauss_table()
    pdf = hist / (hist.sum(-1, keepdims=True) + 1e-6)
    cdf = np.cumsum(pdf, -1)
    return cdf.reshape(B, C, BINS).astype(np.float32)


def run_device(x, trace=False):
    nc = _get_compiled()
    in_maps = [{"xc": s} for s in _shard_x(np.asarray(x))]
    res = run_bass_kernel_spmd(nc, in_maps, list(range(NCORES)), trace=trace)
    tables = [res.results[i]["table"] for i in range(NCORES)]
    return tables, res


def kernel(x, centers):
    # centers is linspace(0,1,256) by construction; bin geometry is hardcoded.
    tables, _ = run_device(x)
    return _postprocess(tables)


if __name__ == "__main__":
    import jax, jax.numpy as jnp
    key = jax.random.key(0)
    k1, _ = jax.random.split(key)
    x = np.asarray(jax.random.uniform(k1, (B, C, H, W), dtype=jnp.float32))
    centers = np.linspace(0, 1, BINS, dtype=np.float32)
    out = kernel(x, centers)
    print("kernel output", out.shape, out.dtype, out[0, 0, :5], out[0, 0, -1])


# revision 13
# speedup vs baseline: 2.4147x; 1.0852x over previous
"""Trainium2 Bass kernel for nn_DifferentiableCDF (soft Gaussian histogram -> CDF).

Exact integer-bin histogram on device, Gaussian applied on host:
  The Gaussian soft-binning weight exp(-((255x - j)/2.55)^2) varies slowly
  enough that quantizing each pixel to its nearest integer bin m = RNE(255x)
  and applying the Gaussian as a 256x256 host-side table multiply changes the
  final CDF by < 2e-4 (vs 2e-2 tolerance).  The device kernel only builds an
  EXACT 256-bin histogram per (B,C) unit, factored as a 16x16 joint count
  table F[hi, s] (hi = m>>4, s = m&15) via one-hot x one-hot matmuls:
    - ACT: m = RNE(255*x + 1024) in fp16 (fp16 is integer-exact in [1024,2048))
    - DVE: hi = RNE(m/16 - 64.4995) (int32 RNE convert), s = m - 16*hi
    - DVE: 16-wide one-hots of hi and s via two broadcast tensor_tensor
      is_equal ops against constant index tiles (4x perf mode)
    - PE:  8 chunks (8*128 pixels) per matmul: the one-hots are laid out
      [128, cb, w=16, cc=8] so each weight block [:, cb, :, :] is 128
      contiguous columns; out[8w+cc, 8s+cc'] accumulates in PSUM, the
      cc-diagonal holds the per-chunk 16x16 tables (96 matmuls total).
  PSUM trick: banks are DVE-memset to zero and all matmuls use start=False so
  per-element has_written semantics accumulate correctly without bank-wide
  clears (which would corrupt interleaved accumulation groups).
  Host sums 8 cores, extracts the cc-diagonal, applies the Gaussian table,
  normalizes and cumsums (output is only 12x256).
"""
import sys
if "/opt/trn_rl_repo" not in sys.path:
    sys.path.insert(0, "/opt/trn_rl_repo")

import numpy as np
from concourse import bacc, tile
from concourse.bass_utils import run_bass_kernel_spmd
import concourse.mybir as mybir

# ---- problem constants (hardcoded per spec) ----
B, C, H, W = 4, 3, 256, 256
UNITS = B * C                  # 12 independent histograms
NPIX = H * W                   # 65536 pixels per unit
NCORES = 8
PIX_PER_CORE = NPIX // NCORES  # 8192 pixels per unit per core
CHUNKS_PER_UNIT = PIX_PER_CORE // 128  # 64
NCHUNK = UNITS * CHUNKS_PER_UNIT       # 768 chunks of 128 pixels
CB = NCHUNK // 8                       # 96 8-chunk superblocks
SIGMA = 0.01
BINS = 256
ALPHA = 1.0 / (255.0 * SIGMA) ** 2     # Gaussian exponent per bin^2
NB = 3                                 # PSUM banks (4 units each)
DT = mybir.dt

_COMPILED = None  # cached nc


def _emit_body(nc, tc, pool, pipe, psum_pool, const_hi, const_s, x_ext,
               tbl_ext, emit_oh=True, emit_mm=True):
    xc = pipe.tile([128, CB, 8], DT.float32, tag="xc")
    nc.sync.dma_start(xc[:], x_ext[:])

    m_h = pipe.tile([128, CB, 8], DT.float16, tag="mh")
    hm_h = pipe.tile([128, CB, 8], DT.float16, tag="hmh")
    s_h = pipe.tile([128, CB, 8], DT.float16, tag="sh")

    # m = RNE(255x + 1024); fp16 grid spacing is exactly 1.0 in [1024, 2048)
    nc.scalar.activation(m_h[:], xc[:], mybir.ActivationFunctionType.Copy,
                         bias=1024.0, scale=255.0)
    # s = m mod 16 (1024 = 0 mod 16), in [0, 16)
    nc.vector.tensor_scalar(s_h[:], m_h[:], 16.0, None, mybir.AluOpType.mod)
    # hm = m - s = 1024 + 16*hi; one-hot of hi compares against 1024+16w
    nc.vector.scalar_tensor_tensor(hm_h[:], s_h[:], -1.0, m_h[:],
                                   mybir.AluOpType.mult, mybir.AluOpType.add)

    oh_hi = pipe.tile([128, CB, 16, 8], DT.float16, tag="ohhi")
    oh_s = pipe.tile([128, CB, 16, 8], DT.float16, tag="ohs")
    if emit_oh:
        hv = hm_h[:].unsqueeze(2).broadcast_to((128, CB, 16, 8))
        sv = s_h[:].unsqueeze(2).broadcast_to((128, CB, 16, 8))
        nc.vector.tensor_tensor(oh_hi[:], hv, const_hi[:],
                                mybir.AluOpType.is_equal)
        nc.vector.tensor_tensor(oh_s[:], sv, const_s[:],
                                mybir.AluOpType.is_equal)
    else:
        nc.vector.memset(oh_hi[:, 0, 0, :], 0.0)
        nc.vector.memset(oh_s[:, 0, 0, :], 0.0)

    accs = []
    for i in range(NB):
        acc_i = psum_pool.tile([128, 512], DT.float32, tag=f"acc{i}")
        accs.append(acc_i)

    if emit_mm:
        for t in range(UNITS):
            a = accs[t // 4]
            col = (t % 4) * 128
            for jj in range(CHUNKS_PER_UNIT // 8):
                cb = t * (CHUNKS_PER_UNIT // 8) + jj
                # start=True on a bank's first matmul clears the whole
                # bank's has_written bits; later units in the bank rely on
                # per-element overwrite-then-accumulate semantics.
                nc.tensor.matmul(a[:, col:col + 128],
                                 oh_hi[:, cb, :, :].rearrange(
                                     "p a b -> p (a b)"),
                                 oh_s[:, cb, :, :].rearrange(
                                     "p a b -> p (a b)"),
                                 start=(t % 4 == 0 and jj == 0), stop=False,
                                 skip_group_check=True)
    else:
        for a in accs:
            nc.vector.memset(a[:], 0.0)

    out_sb = pipe.tile([128, NB * 512], DT.float32, tag="osb")
    for i, a in enumerate(accs):
        nc.scalar.activation(out_sb[:, i * 512:(i + 1) * 512], a[:],
                             mybir.ActivationFunctionType.Copy)
    nc.sync.dma_start(tbl_ext[:], out_sb[:])


def _build(loop_n=1, emit_oh=True, emit_mm=True):
    nc = bacc.Bacc("TRN2", target_bir_lowering=False, debug=False,
                   num_devices=NCORES)
    x_ext = nc.declare_dram_parameter("xc", [128, NCHUNK], DT.float32,
                                      isOutput=False)
    tbl_ext = nc.declare_dram_parameter("table", [128, NB * 512], DT.float32,
                                        isOutput=True)

    with tile.TileContext(nc) as tc:
        with (
            tc.tile_pool(name="pool", bufs=1) as pool,
            tc.tile_pool(name="pipe", bufs=2) as pipe,
            tc.tile_pool(name="psum", bufs=2, space="PSUM") as psum_pool,
        ):
            # loop-invariant one-hot comparison index tiles
            const_hi = pool.tile([128, CB, 16, 8], DT.float16)
            const_s = pool.tile([128, CB, 16, 8], DT.float16)
            for w in range(16):
                nc.vector.memset(const_hi[:, :, w, :], float(w))
                nc.vector.memset(const_s[:, :, w, :], float(1024 + w))

            if loop_n == 1:
                _emit_body(nc, tc, pool, pipe, psum_pool, const_hi, const_s,
                           x_ext, tbl_ext, emit_oh, emit_mm)
            else:
                engs = [mybir.EngineType.PE, mybir.EngineType.DVE,
                        mybir.EngineType.Activation, mybir.EngineType.SP]
                with tc.For_i(0, loop_n, 1, hint_engines=engs):
                    _emit_body(nc, tc, pool, pipe, psum_pool, const_hi,
                               const_s, x_ext, tbl_ext, emit_oh, emit_mm)

    nc.compile()
    return nc


def _get_compiled():
    global _COMPILED
    if _COMPILED is None:
        _COMPILED = _build()
    return _COMPILED


def _shard_x(x):
    """x (B,C,H,W) -> per-core [128, NCHUNK] arrays; element [p, 64t+j] =
    unit t, pixel 8192*core + 128*j + p."""
    xu = np.ascontiguousarray(x.reshape(UNITS, NPIX))
    shards = []
    for core in range(NCORES):
        sl = xu[:, core * PIX_PER_CORE:(core + 1) * PIX_PER_CORE]
        # (UNITS, 64, 128) -> (128, UNITS, 64)
        sl = sl.reshape(UNITS, CHUNKS_PER_UNIT, 128).transpose(2, 0, 1)
        shards.append(np.ascontiguousarray(sl.reshape(128, NCHUNK), np.float32))
    return shards


_GTAB = None


def _gauss_table():
    global _GTAB
    if _GTAB is None:
        m = np.arange(BINS, dtype=np.float64)
        _GTAB = np.exp(-ALPHA * (m[:, None] - m[None, :]) ** 2)
    return _GTAB


def _postprocess(tables):
    """tables: list of NCORES arrays [128, NB*512] -> cdf (B, C, BINS)."""
    tab = np.zeros((128, NB * 512), np.float64)
    for t in tables:
        tab += t.astype(np.float64)
    F = np.zeros((UNITS, 16, 16), np.float64)
    for t in range(UNITS):
        col = (t // 4) * 512 + (t % 4) * 128
        blk = tab[:, col:col + 128].reshape(16, 8, 16, 8)  # [w, cc, s, cc']
        F[t] = np.einsum('wasa->ws', blk)
    F = F.reshape(UNITS, BINS)            # m = 16*hi + s
    hist = F @ _gauss_table()
    pdf = hist / (hist.sum(-1, keepdims=True) + 1e-6)
    cdf = np.cumsum(pdf, -1)
    return cdf.reshape(B, C, BINS).astype(np.float32)


def run_device(x, trace=False):
    nc = _get_compiled()
    in_maps = [{"xc": s} for s in _shard_x(np.asarray(x))]
    res = run_bass_kernel_spmd(nc, in_maps, list(range(NCORES)), trace=trace)
    tables = [res.results[i]["table"] for i in range(NCORES)]
    return tables, res


def kernel(x, centers):
    # centers is linspace(0,1,256) by construction; bin geometry is hardcoded.
    tables, _ = run_device(x)
    return _postprocess(tables)


if __name__ == "__main__":
    import jax, jax.numpy as jnp
    key = jax.random.key(0)
    k1, _ = jax.random.split(key)
    x = np.asarray(jax.random.uniform(k1, (B, C, H, W), dtype=jnp.float32))
    centers = np.linspace(0, 1, BINS, dtype=np.float32)
    out = kernel(x, centers)
    print("kernel output", out.shape, out.dtype, out[0, 0, :5], out[0, 0, -1])


# revision 14
# speedup vs baseline: 12.0837x; 5.0043x over previous
"""Trainium2 Bass kernel for nn_DifferentiableCDF (soft Gaussian histogram -> CDF).

Exact integer-bin histogram on device, Gaussian applied on host:
  The Gaussian soft-binning weight exp(-((255x - j)/2.55)^2) varies slowly
  enough that quantizing each pixel to its nearest integer bin m = RNE(255x)
  and applying the Gaussian as a 256x256 host-side table multiply changes the
  final CDF by < 2e-4 (vs 2e-2 tolerance).  The device kernel only builds an
  EXACT 256-bin histogram per (B,C) unit, factored as a 16x16 joint count
  table F[hi, s] (hi = m>>4, s = m&15) via one-hot x one-hot matmuls:
    - ACT: m = RNE(255*x + 1024) in fp16 (fp16 is integer-exact in [1024,2048))
    - DVE: hi = RNE(m/16 - 64.4995) (int32 RNE convert), s = m - 16*hi
    - DVE: 16-wide one-hots of hi and s via two broadcast tensor_tensor
      is_equal ops against constant index tiles (4x perf mode)
    - PE:  8 chunks (8*128 pixels) per matmul: the one-hots are laid out
      [128, cb, w=16, cc=8] so each weight block [:, cb, :, :] is 128
      contiguous columns; out[8w+cc, 8s+cc'] accumulates in PSUM, the
      cc-diagonal holds the per-chunk 16x16 tables (96 matmuls total).
  PSUM trick: banks are DVE-memset to zero and all matmuls use start=False so
  per-element has_written semantics accumulate correctly without bank-wide
  clears (which would corrupt interleaved accumulation groups).
  Host sums 8 cores, extracts the cc-diagonal, applies the Gaussian table,
  normalizes and cumsums (output is only 12x256).
"""
import sys
if "/opt/trn_rl_repo" not in sys.path:
    sys.path.insert(0, "/opt/trn_rl_repo")

import numpy as np
from concourse import bacc, tile
from concourse.bass_utils import run_bass_kernel_spmd
import concourse.mybir as mybir

# ---- problem constants (hardcoded per spec) ----
B, C, H, W = 4, 3, 256, 256
UNITS = B * C                  # 12 independent histograms
NPIX = H * W                   # 65536 pixels per unit
NCORES = 8
PIX_PER_CORE = NPIX // NCORES  # 8192 pixels per unit per core
CHUNKS_PER_UNIT = PIX_PER_CORE // 128  # 64
NCHUNK = UNITS * CHUNKS_PER_UNIT       # 768 chunks of 128 pixels
CB = NCHUNK // 8                       # 96 8-chunk superblocks
SIGMA = 0.01
BINS = 256
ALPHA = 1.0 / (255.0 * SIGMA) ** 2     # Gaussian exponent per bin^2
NB = 3                                 # PSUM banks (4 units each)
DT = mybir.dt

_COMPILED = None  # cached nc


def _emit_body(nc, tc, pool, pipe, psum_pool, const_hi, const_s, x_ext,
               tbl_ext, emit_oh=True, emit_mm=True):
    xc = pipe.tile([128, CB, 8], DT.float32, tag="xc")
    nc.sync.dma_start(xc[:], x_ext[:])

    m_h = pipe.tile([128, CB, 8], DT.float16, tag="mh")
    hi_i = pipe.tile([128, CB, 8], DT.int32, tag="hii")
    hi_h = pipe.tile([128, CB, 8], DT.float16, tag="hih")
    s_h = pipe.tile([128, CB, 8], DT.float16, tag="sh")

    # m = RNE(255x + 1024); fp16 grid spacing is exactly 1.0 in [1024, 2048)
    nc.scalar.activation(m_h[:], xc[:], mybir.ActivationFunctionType.Copy,
                         bias=1024.0, scale=255.0)
    # hi = floor((m-1024)/16) via RNE(m/16 - 64.4995); int32 convert is RNE
    nc.vector.tensor_scalar(hi_i[:], m_h[:], 0.0625, -64.4995,
                            mybir.AluOpType.mult, mybir.AluOpType.add)
    nc.vector.tensor_copy(hi_h[:], hi_i[:])
    # s = m - 16*hi  in [1024, 1039]
    nc.vector.scalar_tensor_tensor(s_h[:], hi_h[:], -16.0, m_h[:],
                                   mybir.AluOpType.mult, mybir.AluOpType.add)

    oh_hi = pipe.tile([128, CB, 16, 8], DT.float16, tag="ohhi")
    oh_s = pipe.tile([128, CB, 16, 8], DT.float16, tag="ohs")
    if emit_oh:
        hv = hi_h[:].unsqueeze(2).broadcast_to((128, CB, 16, 8))
        sv = s_h[:].unsqueeze(2).broadcast_to((128, CB, 16, 8))
        nc.vector.tensor_tensor(oh_hi[:], hv, const_hi[:],
                                mybir.AluOpType.is_equal)
        nc.vector.tensor_tensor(oh_s[:], sv, const_s[:],
                                mybir.AluOpType.is_equal)
    else:
        nc.vector.memset(oh_hi[:, 0, 0, :], 0.0)
        nc.vector.memset(oh_s[:, 0, 0, :], 0.0)

    accs = []
    for i in range(NB):
        acc_i = psum_pool.tile([128, 512], DT.float32, tag=f"acc{i}")
        accs.append(acc_i)

    if emit_mm:
        for t in range(UNITS):
            a = accs[t // 4]
            col = (t % 4) * 128
            for jj in range(CHUNKS_PER_UNIT // 8):
                cb = t * (CHUNKS_PER_UNIT // 8) + jj
                # start=True on a bank's first matmul clears the whole
                # bank's has_written bits; later units in the bank rely on
                # per-element overwrite-then-accumulate semantics.
                nc.tensor.matmul(a[:, col:col + 128],
                                 oh_hi[:, cb, :, :].rearrange(
                                     "p a b -> p (a b)"),
                                 oh_s[:, cb, :, :].rearrange(
                                     "p a b -> p (a b)"),
                                 start=(t % 4 == 0 and jj == 0), stop=False,
                                 skip_group_check=True)
    else:
        for a in accs:
            nc.vector.memset(a[:], 0.0)

    out_sb = pipe.tile([128, NB * 512], DT.float32, tag="osb")
    for i, a in enumerate(accs):
        nc.scalar.activation(out_sb[:, i * 512:(i + 1) * 512], a[:],
                             mybir.ActivationFunctionType.Copy)
    nc.sync.dma_start(tbl_ext[:], out_sb[:])


def _build(loop_n=1, emit_oh=True, emit_mm=True):
    nc = bacc.Bacc("TRN2", target_bir_lowering=False, debug=False,
                   num_devices=NCORES)
    x_ext = nc.declare_dram_parameter("xc", [128, NCHUNK], DT.float32,
                                      isOutput=False)
    tbl_ext = nc.declare_dram_parameter("table", [128, NB * 512], DT.float32,
                                        isOutput=True)

    with tile.TileContext(nc) as tc:
        with (
            tc.tile_pool(name="pool", bufs=1) as pool,
            tc.tile_pool(name="pipe", bufs=2) as pipe,
            tc.tile_pool(name="psum", bufs=2, space="PSUM") as psum_pool,
        ):
            # loop-invariant one-hot comparison index tiles
            const_hi = pool.tile([128, CB, 16, 8], DT.float16)
            const_s = pool.tile([128, CB, 16, 8], DT.float16)
            for w in range(16):
                nc.vector.memset(const_hi[:, :, w, :], float(w))
                nc.vector.memset(const_s[:, :, w, :], float(1024 + w))

            if loop_n == 1:
                _emit_body(nc, tc, pool, pipe, psum_pool, const_hi, const_s,
                           x_ext, tbl_ext, emit_oh, emit_mm)
            else:
                engs = [mybir.EngineType.PE, mybir.EngineType.DVE,
                        mybir.EngineType.Activation, mybir.EngineType.SP]
                with tc.For_i(0, loop_n, 1, hint_engines=engs):
                    _emit_body(nc, tc, pool, pipe, psum_pool, const_hi,
                               const_s, x_ext, tbl_ext, emit_oh, emit_mm)

    nc.compile()
    return nc


def _get_compiled():
    global _COMPILED
    if _COMPILED is None:
        _COMPILED = _build()
    return _COMPILED


def _shard_x(x):
    """x (B,C,H,W) -> per-core [128, NCHUNK] arrays; element [p, 64t+j] =
    unit t, pixel 8192*core + 128*j + p."""
    xu = np.ascontiguousarray(x.reshape(UNITS, NPIX))
    shards = []
    for core in range(NCORES):
        sl = xu[:, core * PIX_PER_CORE:(core + 1) * PIX_PER_CORE]
        # (UNITS, 64, 128) -> (128, UNITS, 64)
        sl = sl.reshape(UNITS, CHUNKS_PER_UNIT, 128).transpose(2, 0, 1)
        shards.append(np.ascontiguousarray(sl.reshape(128, NCHUNK), np.float32))
    return shards


_GTAB = None


def _gauss_table():
    global _GTAB
    if _GTAB is None:
        m = np.arange(BINS, dtype=np.float64)
        _GTAB = np.exp(-ALPHA * (m[:, None] - m[None, :]) ** 2)
    return _GTAB


def _postprocess(tables):
    """tables: list of NCORES arrays [128, NB*512] -> cdf (B, C, BINS)."""
    tab = np.zeros((128, NB * 512), np.float64)
    for t in tables:
        tab += t.astype(np.float64)
    F = np.zeros((UNITS, 16, 16), np.float64)
    for t in range(UNITS):
        col = (t // 4) * 512 + (t % 4) * 128
        blk = tab[:, col:col + 128].reshape(16, 8, 16, 8)  # [w, cc, s, cc']
        F[t] = np.einsum('wasa->ws', blk)
    F = F.reshape(UNITS, BINS)            # m = 16*hi + s
    hist = F @ _gauss_table()
    pdf = hist / (hist.sum(-1, keepdims=True) + 1e-6)
    cdf = np.cumsum(pdf, -1)
    return cdf.reshape(B, C, BINS).astype(np.float32)


def run_device(x, trace=False):
    nc = _get_compiled()
    in_maps = [{"xc": s} for s in _shard_x(np.asarray(x))]
    res = run_bass_kernel_spmd(nc, in_maps, list(range(NCORES)), trace=trace)
    tables = [res.results[i]["table"] for i in range(NCORES)]
    return tables, res


def kernel(x, centers):
    # centers is linspace(0,1,256) by construction; bin geometry is hardcoded.
    tables, _ = run_device(x)
    return _postprocess(tables)


if __name__ == "__main__":
    import jax, jax.numpy as jnp
    key = jax.random.key(0)
    k1, _ = jax.random.split(key)
    x = np.asarray(jax.random.uniform(k1, (B, C, H, W), dtype=jnp.float32))
    centers = np.linspace(0, 1, BINS, dtype=np.float32)
    out = kernel(x, centers)
    print("kernel output", out.shape, out.dtype, out[0, 0, :5], out[0, 0, -1])
